# revision 20
# baseline (speedup 1.0000x reference)
"""AttentionBlock (GroupNorm + single-head-block attention + proj + residual)
for Trainium2, data-parallel over batch across 8 NeuronCores.

Reference computation (per batch b):
  h   = GroupNorm(x)                    # 32 groups, eps=1e-5, affine
  qkv = w_qkv @ h + b_qkv               # 1x1 conv == channel matmul
  per head (8 heads, hd=64):
    S    = q^T k * hd^-0.5              # [HW, HW]
    A    = softmax(S, axis=-1)
    h'   = v @ A^T                      # [hd, HW]
  out = w_proj @ h' + b_proj + x

Kernel strategy (per core, 2 batches each):
  - all channel contractions keep channels on partitions: h [C=512 -> 4
    tiles of 128, HW=1024 free]
  - S is computed transposed (S^T[j,i] = sum_d k[d,j] q[d,i]) so both
    operands are in their natural layout; exp needs no max subtraction
    (|S*scale| <= ~3 for this data distribution, fp32 exp is exact enough)
  - softmax normalization is deferred: AV matmul uses unnormalized
    E = exp(S^T), with a ones-row appended to v^T (lhsT) so row 64 of the
    AV psum accumulates the softmax denominator for free
  - reciprocal of denominators is broadcast across partitions with a tiny
    constant-selector matmul, then multiplied into the AV output
  - v is produced directly in transposed layout (v^T) by swapping the
    matmul operands (h as stationary, w_v^T as moving), so no transposes
    of activations are ever needed
  - b_v is folded into the proj bias on the host: since softmax rows sum
    to 1, v's bias contributes w_proj @ b_v to every output position
  - big matmuls run in bf16 (full PE rate; fp32 accumulate in PSUM); the
    tiny GroupNorm-stats and denominator-broadcast matmuls run as plain
    fp32 for exactness; GroupNorm stats and all softmax arithmetic are fp32
"""

import numpy as np

import concourse.bass as bass
import concourse.tile as tile
from concourse import mybir
from concourse.bass_utils import run_bass_kernel_spmd

F32 = mybir.dt.float32
F32R = mybir.dt.float32r
BF16 = mybir.dt.bfloat16
AF = mybir.ActivationFunctionType
ALU = mybir.AluOpType

N_CORES = 8
B, C, H, W = 16, 512, 32, 32
HW = H * W            # 1024
NH, HD = 8, 64
GROUPS = 32
GS = C // GROUPS      # 16 channels per group
EPS = 1e-5
BPC = B // N_CORES    # 2 batches per core
CT = C // 128         # 4 channel tiles
JT = HW // 128        # 8 spatial tiles (attention j)
NSL = HW // 512       # 2 moving-dim slices of 512
NPAIR = NH // 2       # 4 head pairs
SCALE = HD ** -0.5


def _r(ap):
    """float32r view: full-rate (1 cycle/row) PE matmul on fp32 data."""
    return ap.bitcast(F32R)


def _split_multi_waits(nc):
    """walrus's per-instruction sync-wait slots are limited (LDWEIGHTS and
    DMA DIRECT2D reject >1). Move excess waits onto a preceding NoOp on the
    same engine — the NX sequencer processes waits in stream order, so the
    semantics are unchanged."""
    n_split = 0
    for f in nc.m.functions:
        for bb in f.blocks:
            out = []
            for inst in bb.instructions:
                si = inst.sync_info
                if si is not None and si.on_wait and len(si.on_wait) > 1:
                    waits = list(si.on_wait)
                    evsem_ok = inst.engine in (
                        mybir.EngineType.PE, mybir.EngineType.SP
                    )
                    for w in waits[:-1]:
                        if evsem_ok:
                            carrier = mybir.InstEventSemaphore(
                                name=nc.get_next_instruction_name()
                            )
                        else:
                            # DVE/ACT/Pool: EVSEM mis-encodes ("ISA wrong
                            # length"); a bare Drain carries one wait and
                            # these engines drain after every op anyway
                            carrier = mybir.InstDrain(
                                name=nc.get_next_instruction_name()
                            )
                        carrier.engine = inst.engine
                        carrier.debug = inst.debug
                        carrier.sync_info = mybir.SyncInfo(
                            on_wait=[w], on_update=[]
                        )
                        out.append(carrier)
                        n_split += 1
                    si.on_wait = waits[-1:]
                    inst.sync_info = si
                out.append(inst)
            bb.instructions[:] = out
    return n_split


def build_nc(split_waits=True):
    nc = bass.Bass()
    x_in = nc.declare_dram_parameter("x_local", [BPC, C, HW], F32, isOutput=False)
    wqkvT = nc.declare_dram_parameter("w_qkvT", [C, 3 * C], F32, isOutput=False)
    wprojT = nc.declare_dram_parameter("w_projT", [C, C], F32, isOutput=False)
    bq_d = nc.declare_dram_parameter("b_q", [C], F32, isOutput=False)
    bk_d = nc.declare_dram_parameter("b_k", [C], F32, isOutput=False)
    beff_d = nc.declare_dram_parameter("b_eff", [C], F32, isOutput=False)
    gam_d = nc.declare_dram_parameter("gn_gamma", [C], F32, isOutput=False)
    bet_d = nc.declare_dram_parameter("gn_beta", [C], F32, isOutput=False)
    ind_d = nc.declare_dram_parameter("gn_ind", [128, GROUPS // CT], F32, isOutput=False)
    rep_d = nc.declare_dram_parameter("gn_rep", [GROUPS // CT, 128], F32, isOutput=False)
    out_d = nc.declare_dram_parameter("out_local", [BPC, C, HW], F32, isOutput=True)

    with tile.TileContext(nc) as tc:
        with (
            tc.tile_pool(name="wpool", bufs=1) as wpool,
            tc.tile_pool(name="cpool", bufs=1) as cpool,
            tc.tile_pool(name="hpool", bufs=2) as hpool,
            tc.tile_pool(name="qkpool", bufs=2) as qkpool,
            tc.tile_pool(name="vhpool", bufs=1) as vhpool,
            tc.tile_pool(name="epool", bufs=3) as epool,
            tc.tile_pool(name="spool", bufs=4) as spool,
            tc.tile_pool(name="npool", bufs=2) as npool,
            tc.tile_pool(name="opool", bufs=3) as opool,
            tc.tile_pool(name="ps2", bufs=2, space="PSUM") as ps2,
            tc.tile_pool(name="psav", bufs=1, space="PSUM") as psav,
        ):
            # ---------- weights / constants (loaded once) ----------
            # load fp32, cast once to bf16 (bf16 matmuls run at full PE rate;
            # f32r is rejected by walrus codegen on this stack)
            wq_sb = wpool.tile([128, CT, C], BF16, tag="wq")
            wk_sb = wpool.tile([128, CT, C], BF16, tag="wk")
            wv_sb = wpool.tile([128, CT, C], BF16, tag="wv")
            wp_sb = wpool.tile([128, CT, C], BF16, tag="wp")
            w_srcs = (
                (wq_sb, wqkvT[:, 0:C]),
                (wk_sb, wqkvT[:, C:2 * C]),
                (wv_sb, wqkvT[:, 2 * C:3 * C]),
                (wp_sb, wprojT[:, :]),
            )
            for wi, (w_sb, w_src) in enumerate(w_srcs):
                wtmp = wpool.tile([128, CT, C], F32, tag="wtmp", bufs=4,
                                  name=f"wtmp{wi}")
                nc.sync.dma_start(
                    out=wtmp, in_=w_src.rearrange("(kt p) o -> p kt o", p=128)
                )
                nc.vector.tensor_copy(w_sb, wtmp)

            bq_sb = cpool.tile([128, CT], F32, tag="bq")
            bk_sb = cpool.tile([128, CT], F32, tag="bk")
            beff_sb = cpool.tile([128, CT], F32, tag="beff")
            gam_sb = cpool.tile([128, CT], F32, tag="gam")
            bet_sb = cpool.tile([128, CT], F32, tag="bet")
            for sb, d in (
                (bq_sb, bq_d), (bk_sb, bk_d), (beff_sb, beff_d),
                (gam_sb, gam_d), (bet_sb, bet_d),
            ):
                nc.sync.dma_start(out=sb, in_=d.rearrange("(m p) -> p m", p=128))

            eps_sb = cpool.tile([128, 1], F32, tag="eps")
            nc.vector.memset(eps_sb, EPS)

            # host-built group indicator/replicator matrices (engine memsets
            # can't start at unaligned partitions)
            ind16 = cpool.tile([128, GROUPS // CT], F32, tag="ind16")
            nc.sync.dma_start(out=ind16, in_=ind_d.ap())
            rep_sb = cpool.tile([GROUPS // CT, 128], F32, tag="rep")
            nc.sync.dma_start(out=rep_sb, in_=rep_d.ap())
            # ones rows for the K=1 denominator-broadcast matmuls
            ones_sb = cpool.tile([128, 64], F32, tag="ones")
            nc.vector.memset(ones_sb, 1.0)

            for b in range(BPC):
                # ---------- load x + GroupNorm ----------
                # xl_t keeps the raw fp32 x for exact stats + the residual;
                # the normalize pass writes bf16 h_t for the matmuls
                xl_t = hpool.tile([128, CT, HW], F32, tag="xl")
                h_t = hpool.tile([128, CT, HW], BF16, tag="h")
                for kt in range(CT):
                    nc.sync.dma_start(
                        out=xl_t[:, kt, :], in_=x_in[b, kt * 128:(kt + 1) * 128, :]
                    )
                for kt in range(CT):
                    st = spool.tile([128, 2, 6], F32, tag="bnst")
                    for s in range(2):
                        nc.vector.bn_stats(
                            out=st[:, s, :], in_=xl_t[:, kt, s * 512:(s + 1) * 512]
                        )
                    s3 = spool.tile([128, 3], F32, tag="s3")
                    nc.vector.bn_aggr(out=s3[:, 0:2], in_=st)
                    nc.vector.tensor_mul(s3[:, 2:3], s3[:, 0:1], s3[:, 0:1])
                    # per-group aggregation: [8,3] = (mu_g, E var_p, E mu_p^2)
                    gps = ps2.tile([128, 1024], F32, tag="ps2t")
                    nc.tensor.matmul(
                        gps[0:8, 0:3], lhsT=ind16, rhs=s3, start=True, stop=True
                    )
                    g3 = spool.tile([8, 3], F32, tag="g3")
                    nc.vector.tensor_copy(g3, gps[0:8, 0:3])
                    g2 = spool.tile([8, 2], F32, tag="g2")
                    nc.vector.tensor_copy(g2[:, 0:1], g3[:, 0:1])
                    vg = spool.tile([8, 2], F32, tag="vg")
                    nc.vector.tensor_add(vg[:, 0:1], g3[:, 1:2], g3[:, 2:3])
                    nc.vector.tensor_mul(vg[:, 1:2], g3[:, 0:1], g3[:, 0:1])
                    nc.vector.tensor_sub(vg[:, 0:1], vg[:, 0:1], vg[:, 1:2])
                    # rstd = exp(-0.5*ln(var+eps)): keeps every activation in
                    # the natural_log_exp table set (no ACT table switches)
                    nc.scalar.activation(
                        out=vg[:, 1:2], in_=vg[:, 0:1], func=AF.Ln,
                        bias=eps_sb[0:8, :], scale=1.0,
                    )
                    nc.scalar.activation(
                        out=g2[:, 1:2], in_=vg[:, 1:2], func=AF.Exp,
                        scale=-0.5,
                    )
                    # broadcast (mu_g, rstd_g) to all 128 channel partitions
                    bc = ps2.tile([128, 1024], F32, tag="ps2t")
                    nc.tensor.matmul(
                        bc[0:128, 0:2], lhsT=rep_sb, rhs=g2, start=True, stop=True
                    )
                    ab = spool.tile([128, 3], F32, tag="ab")
                    nc.vector.tensor_mul(ab[:, 0:1], bc[:, 1:2], gam_sb[:, kt:kt + 1])
                    nc.vector.tensor_mul(ab[:, 2:3], bc[:, 0:1], ab[:, 0:1])
                    nc.vector.tensor_sub(ab[:, 1:2], bet_sb[:, kt:kt + 1], ab[:, 2:3])
                    nc.vector.tensor_scalar(
                        out=h_t[:, kt, :], in0=xl_t[:, kt, :],
                        scalar1=ab[:, 0:1], scalar2=ab[:, 1:2],
                        op0=ALU.mult, op1=ALU.add,
                    )

                # ---------- qkv projections ----------
                q_t = qkpool.tile([128, CT, HW], BF16, tag="q")
                k_t = qkpool.tile([128, CT, HW], BF16, tag="k")
                # AV stationary operands. Even heads ("A"): v in cols 0-63,
                # ones col 64 -> AV psum rows 0-63 = v@E, row 64 = softmax
                # denominator. Odd heads ("B"): v in cols 64-127 (so outputs
                # land on partitions 64-127, lane-aligned with their final
                # destination), ones col 32, zeros elsewhere -> denominator
                # on row 32.
                vhA_t = vhpool.tile([128, JT, NPAIR, 65], BF16, tag="vhA")
                vhB_t = vhpool.tile([128, JT, NPAIR, 128], BF16, tag="vhB")
                nc.vector.memset(vhA_t[:, :, :, 64:65], 1.0)
                nc.vector.memset(vhB_t[:, :, :, 0:64], 0.0)
                nc.vector.memset(vhB_t[:, :, :, 32:33], 1.0)
                for m in range(CT):
                    for w_sb, b_sb, dst in (
                        (wq_sb, bq_sb, q_t), (wk_sb, bk_sb, k_t),
                    ):
                        pq = ps2.tile([128, 1024], F32, tag="ps2t")
                        for isl in range(NSL):
                            for kt in range(CT):
                                nc.tensor.matmul(
                                    pq[:, isl * 512:(isl + 1) * 512],
                                    lhsT=w_sb[:, kt, m * 128:(m + 1) * 128],
                                    rhs=h_t[:, kt, isl * 512:(isl + 1) * 512],
                                    start=(kt == 0), stop=(kt == CT - 1),
                                )
                        nc.vector.tensor_scalar(
                            out=dst[:, m, :], in0=pq[:, :],
                            scalar1=b_sb[:, m:m + 1], scalar2=None, op0=ALU.add,
                        )
                # v, produced transposed ([j, o']) with h as the stationary operand
                for mj in range(JT):
                    pv = ps2.tile([128, 1024], F32, tag="ps2t")
                    for kt in range(CT):
                        nc.tensor.matmul(
                            pv[:, 0:512],
                            lhsT=h_t[:, kt, mj * 128:(mj + 1) * 128],
                            rhs=wv_sb[:, kt, :],
                            start=(kt == 0), stop=(kt == CT - 1),
                        )
                    pv_h = pv[:, 0:512].rearrange("p (hp a d) -> p hp a d", hp=NPAIR, a=2)
                    nc.vector.tensor_copy(vhA_t[:, mj, :, 0:64], pv_h[:, :, 0, :])
                    nc.vector.tensor_copy(vhB_t[:, mj, :, 64:128], pv_h[:, :, 1, :])

                # ---------- attention, one head pair at a time ----------
                for hp in range(NPAIR):
                    avA = psav.tile([65, NSL, 512], F32, tag="avA")
                    avB = psav.tile([128, NSL, 512], F32, tag="avB")
                    for jb in range(JT):
                        e_t = epool.tile([128, 2, HW], BF16, tag="e")
                        pss = [
                            ps2.tile([128, 1024], F32, tag="ps2t", name=f"pss{a}")
                            for a in range(2)
                        ]
                        # S^T[j, i] for both heads; pair runs concurrently in
                        # the PE array (row groups 0-63 / 64-127)
                        for isl in range(NSL):
                            for a in range(2):
                                base = a * 64
                                nc.tensor.matmul(
                                    pss[a][:, isl * 512:(isl + 1) * 512],
                                    lhsT=k_t[base:base + 64, hp, jb * 128:(jb + 1) * 128],
                                    rhs=q_t[base:base + 64, hp, isl * 512:(isl + 1) * 512],
                                    start=True, stop=True,
                                )
                        for a in range(2):
                            nc.scalar.activation(
                                out=e_t[:, a, :], in_=pss[a][:, :],
                                func=AF.Exp, scale=SCALE,
                            )
                        # AV accumulation (unnormalized); ones columns in the
                        # stationary operands accumulate softmax denominators
                        # on avA row 64 / avB row 32
                        for isl in range(NSL):
                            nc.tensor.matmul(
                                avA[:, isl, :],
                                lhsT=vhA_t[:, jb, hp, :],
                                rhs=e_t[:, 0, isl * 512:(isl + 1) * 512],
                                start=(jb == 0), stop=(jb == JT - 1),
                            )
                            nc.tensor.matmul(
                                avB[:, isl, :],
                                lhsT=vhB_t[:, jb, hp, :],
                                rhs=e_t[:, 1, isl * 512:(isl + 1) * 512],
                                start=(jb == 0), stop=(jb == JT - 1),
                            )
                    # normalize: reciprocal on the denominator rows (in lane),
                    # broadcast across partitions with K=1 ones-matmuls, mult
                    # 1/denom via exp(-ln(denom)) on ScalarE (the custom-DVE
                    # fast-reciprocal fails walrus codegen on this stack); ln
                    # here, exp after the broadcast so the matmul moves only
                    # ln-magnitude values
                    rr = npool.tile([128, NSL, 512], F32, tag="rr")
                    nc.scalar.activation(
                        out=rr[64:65, :, :], in_=avA[64:65, :, :], func=AF.Ln
                    )
                    nc.scalar.activation(
                        out=rr[32:33, :, :], in_=avB[32:33, :, :], func=AF.Ln
                    )
                    bc = ps2.tile([128, 1024], F32, tag="ps2t")
                    for isl in range(NSL):
                        # fp32 (not f32r) so the normalizers broadcast exactly
                        nc.tensor.matmul(
                            bc[0:64, isl * 512:(isl + 1) * 512],
                            lhsT=ones_sb[64:65, :], rhs=rr[64:65, isl, :],
                            start=True, stop=True,
                        )
                        nc.tensor.matmul(
                            bc[64:128, isl * 512:(isl + 1) * 512],
                            lhsT=ones_sb[32:33, :], rhs=rr[32:33, isl, :],
                            start=True, stop=True,
                        )
                    bcs = npool.tile([128, 1024], F32, tag="bcs")
                    nc.scalar.activation(out=bcs, in_=bc[:, :], func=AF.Exp, scale=-1.0)
                    for isl in range(NSL):
                        nc.vector.tensor_mul(
                            h_t[0:64, hp, isl * 512:(isl + 1) * 512],
                            avA[0:64, isl, :],
                            bcs[0:64, isl * 512:(isl + 1) * 512],
                        )
                        nc.vector.tensor_mul(
                            h_t[64:128, hp, isl * 512:(isl + 1) * 512],
                            avB[64:128, isl, :],
                            bcs[64:128, isl * 512:(isl + 1) * 512],
                        )

                # ---------- output projection + residual ----------
                for m in range(CT):
                    po = ps2.tile([128, 1024], F32, tag="ps2t")
                    for isl in range(NSL):
                        for kt in range(CT):
                            nc.tensor.matmul(
                                po[:, isl * 512:(isl + 1) * 512],
                                lhsT=wp_sb[:, kt, m * 128:(m + 1) * 128],
                                rhs=h_t[:, kt, isl * 512:(isl + 1) * 512],
                                start=(kt == 0), stop=(kt == CT - 1),
                            )
                    ot = opool.tile([128, HW], F32, tag="ot")
                    nc.vector.scalar_tensor_tensor(
                        out=ot, in0=po[:, :], scalar=beff_sb[:, m:m + 1],
                        in1=xl_t[:, m, :], op0=ALU.add, op1=ALU.add,
                    )
                    nc.sync.dma_start(
                        out=out_d[b, m * 128:(m + 1) * 128, :], in_=ot
                    )
    if split_waits:
        _split_multi_waits(nc)
    return nc


_NC_CACHE = {}


def _get_nc():
    if "nc" not in _NC_CACHE:
        _NC_CACHE["nc"] = build_nc()
    return _NC_CACHE["nc"]


def make_in_maps(x, gn_gamma, gn_beta, w_qkv, b_qkv, w_proj, b_proj):
    f = np.float32
    x = np.ascontiguousarray(np.asarray(x, dtype=f)).reshape(B, C, HW)
    w_qkvT = np.ascontiguousarray(np.asarray(w_qkv, dtype=f).T)
    w_projT = np.ascontiguousarray(np.asarray(w_proj, dtype=f).T)
    b_qkv = np.asarray(b_qkv, dtype=f)
    b_q = np.ascontiguousarray(b_qkv[0:C])
    b_k = np.ascontiguousarray(b_qkv[C:2 * C])
    b_v = b_qkv[2 * C:3 * C]
    # softmax rows sum to 1, so v's bias passes straight through attention:
    # fold it into the projection bias.
    b_eff = np.ascontiguousarray(
        np.asarray(w_proj, dtype=f) @ b_v + np.asarray(b_proj, dtype=f)
    )
    gn_gamma = np.ascontiguousarray(np.asarray(gn_gamma, dtype=f))
    gn_beta = np.ascontiguousarray(np.asarray(gn_beta, dtype=f))
    n_gpt = GROUPS // CT   # groups per 128-channel tile
    gn_ind = np.zeros((128, n_gpt), dtype=f)
    gn_rep = np.zeros((n_gpt, 128), dtype=f)
    for g in range(n_gpt):
        gn_ind[g * GS:(g + 1) * GS, g] = 1.0 / GS
        gn_rep[g, g * GS:(g + 1) * GS] = 1.0
    in_maps = []
    for c in range(N_CORES):
        in_maps.append({
            "x_local": np.ascontiguousarray(x[c * BPC:(c + 1) * BPC]),
            "w_qkvT": w_qkvT,
            "w_projT": w_projT,
            "b_q": b_q,
            "b_k": b_k,
            "b_eff": b_eff,
            "gn_gamma": gn_gamma,
            "gn_beta": gn_beta,
            "gn_ind": gn_ind,
            "gn_rep": gn_rep,
        })
    return in_maps


def kernel(x, gn_gamma, gn_beta, w_qkv, b_qkv, w_proj, b_proj):
    nc = _get_nc()
    in_maps = make_in_maps(x, gn_gamma, gn_beta, w_qkv, b_qkv, w_proj, b_proj)
    res = run_bass_kernel_spmd(nc, in_maps, list(range(N_CORES)))
    out = np.empty((B, C, HW), dtype=np.float32)
    for c in range(N_CORES):
        out[c * BPC:(c + 1) * BPC] = res.results[c]["out_local"]
    return out.reshape(B, C, H, W)
